# revision 1
# baseline (speedup 1.0000x reference)
"""Chamfer distance loss kernel for Trainium2 (8 NeuronCores).

Strategy
--------
d(n, m) = ||x_n||^2 + ||y_m||^2 - 2 x_n . y_m  is produced directly by the
TensorEngine with a K=5 augmented contraction:
    lhsT rows = [x, y, z, xx, 1]          (predict side, [5, Np])
    rhs  rows = [-2tx, -2ty, -2tz, 1, yy] (target side,  [5, M])
so each matmul emits a [128, 512] tile of the full distance matrix into PSUM.

Sharding: batch b = core//2 on each pair of cores; each core takes half of the
predict rows (4096) and the full 8192-point target set.  Per core:
  * x-direction: row-min over the free dim (min over all targets for each of
    its 4096 predict rows), via a TT-min tree + reduce on the VectorE.
  * y-direction: running elementwise min across row-chunks (col-min partials,
    min over this core's 4096 predict rows), finished on the host with a
    partition-min and a cross-core (pair) min.

The PSUM tiles are evacuated by the ScalarE as bf16.  bf16 rounding is
monotonic, so min over rounded values == rounded true min; the final scalar
only sees ~1e-5 relative error from this while the VectorE gets 2x-mode
throughput on all tensor_tensor mins.
"""

import sys

sys.path.insert(0, "/opt/trn_rl_repo")

import numpy as np

B = 4
N = 8192  # predict points per batch
M = 8192  # target points per batch
NCORES = 8
HALF = N // 2  # predict rows per core (2 cores per batch)

ROW_CHUNKS = HALF // 128  # 32 chunks of 128 predict rows
COL_STRIPS = M // 512  # 16 strips of 512 target cols
STRIP_GROUPS = COL_STRIPS // 4  # 4 groups of 4 strips (one 4-bank PSUM tile)

_CACHE = {}


def _build_nc(repeats=1, acc_bf16=True, gpsimd_jgs=()):
    """Build the SPMD single-core Bass program (same program on all 8 cores).

    repeats: run the main loop this many times (idempotent — used for timing).
    gpsimd_jgs: strip-group indices whose col-min chain runs on GPSIMD.
    """
    import concourse.bass as bass  # noqa: F401
    import concourse.mybir as mybir
    import concourse.tile as tile
    from concourse import bacc

    f32 = mybir.dt.float32
    bf16 = mybir.dt.bfloat16
    acc_dt = bf16 if acc_bf16 else f32
    AluOp = mybir.AluOpType

    nc = bacc.Bacc("TRN2", target_bir_lowering=False, debug=False, num_devices=NCORES)
    lhs_d = nc.dram_tensor("lhs", [5, HALF], f32, kind="ExternalInput")
    rhs_d = nc.dram_tensor("rhs", [5, M], f32, kind="ExternalInput")
    xm_d = nc.dram_tensor("xm", [128, ROW_CHUNKS * STRIP_GROUPS], f32,
                          kind="ExternalOutput")
    ym_d = nc.dram_tensor("ym", [128, M], acc_dt, kind="ExternalOutput")

    with tile.TileContext(nc) as tc:
        with (
            tc.tile_pool(name="persist", bufs=1) as persist,
            tc.tile_pool(name="sbc", bufs=3) as sbc,
            tc.tile_pool(name="tr1", bufs=2) as tr1,
            tc.tile_pool(name="tr2", bufs=2) as tr2,
            tc.tile_pool(name="tr3", bufs=2) as tr3,
            tc.tile_pool(name="psum", bufs=2, space="PSUM") as psum,
        ):
            lhs = persist.tile([5, HALF], f32)
            rhs = persist.tile([5, M], f32)
            acc = persist.tile([128, M], acc_dt)
            rowp = persist.tile([128, ROW_CHUNKS * STRIP_GROUPS], f32)
            nc.gpsimd.dma_start(lhs[:], lhs_d[:])
            nc.gpsimd.dma_start(rhs[:], rhs_d[:])

            for rep in range(repeats):
                for i in range(ROW_CHUNKS):
                    for jg in range(STRIP_GROUPS):
                        pt = psum.tile([128, 2048], f32)
                        for k in range(4):
                            j = jg * 4 + k
                            nc.tensor.matmul(
                                pt[:, k * 512:(k + 1) * 512],
                                lhs[:, i * 128:(i + 1) * 128],
                                rhs[:, j * 512:(j + 1) * 512],
                                start=True,
                                stop=True,
                            )
                        # ScalarE evacuates PSUM -> SBUF (bf16 cast).
                        sb = sbc.tile([128, 2048], acc_dt)
                        nc.scalar.copy(sb[:], pt[:])
                        # Row-min: TT-min tree (2x mode on bf16) + final reduce.
                        t1 = tr1.tile([128, 1024], acc_dt)
                        nc.vector.tensor_tensor(t1[:], sb[:, :1024], sb[:, 1024:],
                                                op=AluOp.min)
                        t2 = tr2.tile([128, 512], acc_dt)
                        nc.vector.tensor_tensor(t2[:], t1[:, :512], t1[:, 512:],
                                                op=AluOp.min)
                        t3 = tr3.tile([128, 256], acc_dt)
                        nc.vector.tensor_tensor(t3[:], t2[:, :256], t2[:, 256:],
                                                op=AluOp.min)
                        s = i * STRIP_GROUPS + jg
                        nc.vector.tensor_reduce(
                            out=rowp[:, s:s + 1], in_=t3[:],
                            axis=mybir.AxisListType.X, op=AluOp.min,
                        )
                        # Col-min running accumulate.
                        a_sl = acc[:, jg * 2048:(jg + 1) * 2048]
                        if i == 0 and rep == 0:
                            nc.vector.tensor_copy(a_sl, sb[:])
                        elif jg in gpsimd_jgs:
                            nc.gpsimd.tensor_tensor(a_sl, sb[:], a_sl, op=AluOp.min)
                        else:
                            nc.vector.tensor_tensor(a_sl, sb[:], a_sl, op=AluOp.min)

            nc.gpsimd.dma_start(xm_d[:], rowp[:])
            nc.gpsimd.dma_start(ym_d[:], acc[:])

    nc.compile()
    return nc


def _get_nc(**kw):
    key = tuple(sorted(kw.items()))
    if key not in _CACHE:
        _CACHE[key] = _build_nc(**kw)
    return _CACHE[key]


def _prep_in_maps(predict, target):
    """Host-side shard + transpose + augment (tiny: ~1.5MB total)."""
    predict = np.asarray(predict, dtype=np.float32)
    target = np.asarray(target, dtype=np.float32)
    in_maps = []
    for c in range(NCORES):
        b, h = divmod(c, 2)
        p = predict[b, h * HALF:(h + 1) * HALF, :]  # [HALF, 3]
        t = target[b]  # [M, 3]
        lhs = np.empty((5, HALF), dtype=np.float32)
        lhs[0:3] = p.T
        lhs[3] = (p * p).sum(axis=1)
        lhs[4] = 1.0
        rhs = np.empty((5, M), dtype=np.float32)
        rhs[0:3] = -2.0 * t.T
        rhs[3] = 1.0
        rhs[4] = (t * t).sum(axis=1)
        in_maps.append({"lhs": np.ascontiguousarray(lhs),
                        "rhs": np.ascontiguousarray(rhs)})
    return in_maps


def _postprocess(results):
    """Combine per-core partials into the scalar loss."""
    xsum = 0.0
    ysum = 0.0
    for b in range(B):
        r0 = results[2 * b]
        r1 = results[2 * b + 1]
        xsum += np.float64(r0["xm"].astype(np.float64)
                           .reshape(128, ROW_CHUNKS, STRIP_GROUPS).min(axis=2).sum())
        xsum += np.float64(r1["xm"].astype(np.float64)
                           .reshape(128, ROW_CHUNKS, STRIP_GROUPS).min(axis=2).sum())
        ym0 = r0["ym"].astype(np.float32).min(axis=0)
        ym1 = r1["ym"].astype(np.float32).min(axis=0)
        ysum += np.minimum(ym0, ym1).astype(np.float64).sum()
    total = xsum / (B * N) + ysum / (B * M)
    return np.float32(total)


def _run(in_maps, **build_kw):
    from concourse.bass_utils import run_bass_kernel_spmd

    nc = _get_nc(**build_kw)
    res = run_bass_kernel_spmd(nc, in_maps, core_ids=list(range(NCORES)))
    return res.results


def kernel(predict, target):
    in_maps = _prep_in_maps(predict, target)
    results = _run(in_maps)
    return _postprocess(results)


if __name__ == "__main__":
    rng = np.random.default_rng(0)
    predict = rng.standard_normal((B, N, 3)).astype(np.float32)
    target = rng.standard_normal((B, M, 3)).astype(np.float32)
    out = kernel(predict, target)
    # numpy reference
    exp_x = 0.0
    exp_y = 0.0
    for b in range(B):
        d = ((predict[b][:, None, :] - target[b][None, :, :]) ** 2).sum(-1)
        exp_x += d.min(axis=1).sum()
        exp_y += d.min(axis=0).sum()
    exp = exp_x / (B * N) + exp_y / (B * M)
    print("kernel:", out, "expected:", exp, "rel err:",
          abs(out - exp) / abs(exp))


# revision 6
# speedup vs baseline: 3.2971x; 3.2971x over previous
"""Chamfer distance loss kernel for Trainium2 (8 NeuronCores).

Strategy
--------
d(n, m) = ||x_n||^2 + ||y_m||^2 - 2 x_n . y_m  is produced directly by the
TensorEngine with a K=5 augmented contraction:
    lhsT rows = [x, y, z, xx, 1]          (predict side, [5, Np])
    rhs  rows = [-2tx, -2ty, -2tz, 1, yy] (target side,  [5, M])
so each matmul emits a [128, 512] tile of the full distance matrix into PSUM.

Sharding: batch b = core//2 on each pair of cores; each core takes half of the
predict rows (4096) and the full 8192-point target set.  Per core:
  * x-direction: row-min over the free dim (min over all targets for each of
    its 4096 predict rows), via a TT-min tree + reduce on the VectorE.
  * y-direction: running elementwise min across row-chunks (col-min partials,
    min over this core's 4096 predict rows), finished on the host with a
    partition-min and a cross-core (pair) min.

The PSUM tiles are evacuated by the ScalarE as bf16.  bf16 rounding is
monotonic, so min over rounded values == rounded true min; the final scalar
only sees ~1e-5 relative error from this while the VectorE gets 2x-mode
throughput on all tensor_tensor mins.
"""

import sys

sys.path.insert(0, "/opt/trn_rl_repo")

import numpy as np

B = 4
N = 8192  # predict points per batch
M = 8192  # target points per batch
NCORES = 8
HALF = N // 2  # predict rows per core (2 cores per batch)

ROW_CHUNKS = HALF // 128  # 32 chunks of 128 predict rows
COL_STRIPS = M // 512  # 16 strips of 512 target cols
STRIP_GROUPS = COL_STRIPS // 4  # 4 groups of 4 strips (one 4-bank PSUM tile)

_CACHE = {}


K_AUG = 24  # 3-way bf16 split: 18 coord rows + 3 xx rows + 3 yy rows


def _build_nc(repeats=1, acc_bf16=True, gps8=0, no_colmin=False,
              no_rowmin=False, sbc_bufs=8, psum_bufs=2, act_evac=True):
    """Build the SPMD single-core Bass program (same program on all 8 cores).

    repeats: run the main loop this many times (idempotent — used for timing).
    gpsimd_jgs: strip-group indices whose col-min chain runs on GPSIMD.
    """
    import concourse.bass as bass  # noqa: F401
    import concourse.mybir as mybir
    import concourse.tile as tile
    from concourse import bacc

    f32 = mybir.dt.float32
    bf16 = mybir.dt.bfloat16
    acc_dt = bf16 if acc_bf16 else f32
    AluOp = mybir.AluOpType

    nc = bacc.Bacc("TRN2", target_bir_lowering=False, debug=False, num_devices=NCORES)
    lhs_d = nc.dram_tensor("lhs", [K_AUG, HALF], bf16, kind="ExternalInput")
    rhs_d = nc.dram_tensor("rhs", [K_AUG, M], bf16, kind="ExternalInput")
    xm_d = nc.dram_tensor("xm", [128, ROW_CHUNKS], f32, kind="ExternalOutput")
    ym_d = nc.dram_tensor("ym", [128, M], acc_dt, kind="ExternalOutput")

    with tile.TileContext(nc) as tc:
        with (
            tc.tile_pool(name="persist", bufs=1) as persist,
            tc.tile_pool(name="sbc", bufs=sbc_bufs) as sbc,
            tc.tile_pool(name="tru", bufs=2) as tru,
            tc.tile_pool(name="trv", bufs=2) as trv,
            tc.tile_pool(name="trw", bufs=2) as trw,
            tc.tile_pool(name="tr1", bufs=2) as tr1,
            tc.tile_pool(name="tr2", bufs=2) as tr2,
            tc.tile_pool(name="tr3", bufs=2) as tr3,
            tc.tile_pool(name="psum", bufs=psum_bufs, space="PSUM") as psum,
        ):
            lhs = persist.tile([K_AUG, HALF], bf16)
            rhs = persist.tile([K_AUG, M], bf16)
            acc = persist.tile([128, M], acc_dt)
            rowp = persist.tile([128, ROW_CHUNKS], f32)
            nc.gpsimd.dma_start(lhs[:], lhs_d[:])
            nc.gpsimd.dma_start(rhs[:], rhs_d[:])

            for rep in range(repeats):
                for i in range(ROW_CHUNKS):
                    sbs = []
                    for jg in range(STRIP_GROUPS):
                        pt = psum.tile([128, 2048], f32)
                        for k in range(4):
                            j = jg * 4 + k
                            nc.tensor.matmul(
                                pt[:, k * 512:(k + 1) * 512],
                                lhs[:, i * 128:(i + 1) * 128],
                                rhs[:, j * 512:(j + 1) * 512],
                                start=True,
                                stop=True,
                            )
                        # ScalarE evacuates PSUM -> SBUF (bf16 cast).
                        sb = sbc.tile([128, 2048], acc_dt)
                        if act_evac:
                            nc.scalar.copy(sb[:], pt[:])
                        else:
                            nc.vector.tensor_copy(sb[:], pt[:])
                        sbs.append(sb)
                        # Col-min running accumulate.
                        if not (no_colmin and not (i == 0 and rep == 0)):
                            a_sl = acc[:, jg * 2048:(jg + 1) * 2048]
                            if i == 0 and rep == 0:
                                nc.vector.tensor_copy(a_sl, sb[:])
                            elif ((i * STRIP_GROUPS + jg) % 8) < gps8:
                                nc.gpsimd.tensor_tensor(a_sl, sb[:], a_sl,
                                                        op=AluOp.min)
                            else:
                                nc.vector.tensor_tensor(a_sl, sb[:], a_sl,
                                                        op=AluOp.min)
                    if not (no_rowmin and i > 0):
                        # Row-min across all 4 strip groups (TT-min tree,
                        # 2x mode on bf16) + final reduce.
                        u = tru.tile([128, 2048], acc_dt)
                        nc.vector.tensor_tensor(u[:], sbs[0][:], sbs[1][:],
                                                op=AluOp.min)
                        v = trv.tile([128, 2048], acc_dt)
                        nc.vector.tensor_tensor(v[:], sbs[2][:], sbs[3][:],
                                                op=AluOp.min)
                        w = trw.tile([128, 2048], acc_dt)
                        nc.vector.tensor_tensor(w[:], u[:], v[:], op=AluOp.min)
                        t1 = tr1.tile([128, 1024], acc_dt)
                        nc.vector.tensor_tensor(t1[:], w[:, :1024], w[:, 1024:],
                                                op=AluOp.min)
                        t2 = tr2.tile([128, 512], acc_dt)
                        nc.vector.tensor_tensor(t2[:], t1[:, :512], t1[:, 512:],
                                                op=AluOp.min)
                        t3 = tr3.tile([128, 256], acc_dt)
                        nc.vector.tensor_tensor(t3[:], t2[:, :256], t2[:, 256:],
                                                op=AluOp.min)
                        nc.vector.tensor_reduce(
                            out=rowp[:, i:i + 1], in_=t3[:],
                            axis=mybir.AxisListType.X, op=AluOp.min,
                        )

            nc.gpsimd.dma_start(xm_d[:], rowp[:])
            nc.gpsimd.dma_start(ym_d[:], acc[:])

    nc.compile()
    return nc


def _get_nc(**kw):
    key = tuple(sorted(kw.items()))
    if key not in _CACHE:
        _CACHE[key] = _build_nc(**kw)
    return _CACHE[key]


def _split3(x):
    """fp32 -> (hi, mid, lo) bf16 triplet with hi+mid+lo ~ x to ~2^-25."""
    import ml_dtypes

    bf = ml_dtypes.bfloat16
    h = x.astype(bf)
    r = x - h.astype(np.float32)
    m = r.astype(bf)
    r2 = r - m.astype(np.float32)
    l = r2.astype(bf)
    return h, m, l


def _prep_in_maps(predict, target):
    """Host-side shard + transpose + augment (tiny: ~3MB total).

    d = sum_k lhs[k,n] * rhs[k,m] reproduces xx + yy - 2 x.y to fp32-level
    accuracy using 3-way bf16 splits: for each scalar product a*b with
    a=ah+am+al, b=bh+bm+bl we keep ah*(bh+bm+bl) + am*(bh+bm) + al*bh;
    dropped terms are O(2^-27).
    """
    import ml_dtypes

    bf = ml_dtypes.bfloat16
    predict = np.asarray(predict, dtype=np.float32)
    target = np.asarray(target, dtype=np.float32)
    in_maps = []
    for c in range(NCORES):
        b, h = divmod(c, 2)
        p = predict[b, h * HALF:(h + 1) * HALF, :]  # [HALF, 3]
        t = target[b]  # [M, 3]
        xx = (p * p).sum(axis=1)
        yy = (t * t).sum(axis=1)
        ph, pm, pl = _split3(p.T)            # [3, HALF] each
        th, tm, tl = _split3(-2.0 * t.T)     # [3, M] each
        xh, xm, xl = _split3(xx[None, :])    # [1, HALF]
        yh, ym, yl = _split3(yy[None, :])    # [1, M]
        one = np.ones((1,), dtype=bf)
        lhs = np.empty((K_AUG, HALF), dtype=bf)
        rhs = np.empty((K_AUG, M), dtype=bf)
        r = 0
        for cd in range(3):  # coordinate products
            # ah*bh, ah*bm, ah*bl, am*bh, am*bm, al*bh
            for a, bb in ((ph, th), (ph, tm), (ph, tl),
                          (pm, th), (pm, tm), (pl, th)):
                lhs[r] = a[cd]
                rhs[r] = bb[cd]
                r += 1
        for a in (xh, xm, xl):  # xx * 1
            lhs[r] = a[0]
            rhs[r] = one
            r += 1
        for bb in (yh, ym, yl):  # 1 * yy
            lhs[r] = one
            rhs[r] = bb[0]
            r += 1
        assert r == K_AUG
        in_maps.append({"lhs": np.ascontiguousarray(lhs),
                        "rhs": np.ascontiguousarray(rhs)})
    return in_maps


def _postprocess(results):
    """Combine per-core partials into the scalar loss."""
    xsum = 0.0
    ysum = 0.0
    for b in range(B):
        r0 = results[2 * b]
        r1 = results[2 * b + 1]
        xsum += float(r0["xm"].astype(np.float64).sum())
        xsum += float(r1["xm"].astype(np.float64).sum())
        ym0 = r0["ym"].astype(np.float32).min(axis=0)
        ym1 = r1["ym"].astype(np.float32).min(axis=0)
        ysum += np.minimum(ym0, ym1).astype(np.float64).sum()
    total = xsum / (B * N) + ysum / (B * M)
    return np.float32(total)


def _run(in_maps, **build_kw):
    from concourse.bass_utils import run_bass_kernel_spmd

    nc = _get_nc(**build_kw)
    res = run_bass_kernel_spmd(nc, in_maps, core_ids=list(range(NCORES)))
    return res.results


def kernel(predict, target):
    in_maps = _prep_in_maps(predict, target)
    results = _run(in_maps)
    return _postprocess(results)


if __name__ == "__main__":
    rng = np.random.default_rng(0)
    predict = rng.standard_normal((B, N, 3)).astype(np.float32)
    target = rng.standard_normal((B, M, 3)).astype(np.float32)
    out = kernel(predict, target)
    # numpy reference
    exp_x = 0.0
    exp_y = 0.0
    for b in range(B):
        d = ((predict[b][:, None, :] - target[b][None, :, :]) ** 2).sum(-1)
        exp_x += d.min(axis=1).sum()
        exp_y += d.min(axis=0).sum()
    exp = exp_x / (B * N) + exp_y / (B * M)
    print("kernel:", out, "expected:", exp, "rel err:",
          abs(out - exp) / abs(exp))
